# revision 50
# baseline (speedup 1.0000x reference)
"""Multi-head attention (B=8, S=1024, D=1024, H=16, dk=dv=64) on 8 TRN2 cores.

Sharding: data-parallel over batch — core b computes batch element b end to
end; no collectives. Host-side prep transposes activations/weights into the
layouts TensorE needs (contraction dim on partitions); all matmuls run on
device in bf16 (fp32 psum accumulate).

Key additions over the previous revision (266us -> 250us):
  * WQ is pre-scaled on host by 16*log2(e), so the scores psum holds
    P = 128*log2(exp-arg) directly. A custom 8-stage DVE op (EXP2BITS)
    converts P to the bf16 BIT PATTERN of exp(score/8)*alpha via a
    Schraudolph-style magic-add (round-to-128 via +-1.5*2^30) with a
    quadratic mantissa correction (max rel err ~0.47%, bf16-grade; out
    dtype uint16 = bf16 bits, round-to-nearest). 7 of 16 exp tiles per
    iteration run on the DVE, relieving the ScalarE activation
    bottleneck (ScalarE handles the rest with a matching *alpha bias —
    exp(x*SC_SCALE + ln(alpha)) — so the common factor cancels in
    softmax normalization). The custom op reads PSUM directly (only
    arithmetic stages — the raw-bit reciprocal op cannot).
  * softmax normalization: recip rows stay in stg; ONE bf16 staging copy,
    then PE rank-1 col+row-tiled matmuls broadcast them into psum and a
    DVE multiply scales ctxT. No gpsimd broadcast / no cross-engine
    convoy (gpsimd tensor ops are slow and library-thrash; a 4-hop
    scalar->DVE->gpsimd->DVE chain was permanently ~1 iteration behind
    and head-of-line blocked the DVE FIFO).
  * fc tail: bf16 fc partials are accumulated into psum via an identity
    matmul (PE has slack in the drain) instead of DVE adds; evictions
    split/alternate DVE+ScalarE; output is stored bf16 (host casts back
    to fp32) halving the output-DMA drain; stores alternate sync/scalar
    queues; fc ct4-6 emission is split around pv(7) as PE filler.
  * startup: first v-tiles as single whole-tile descriptors (queue issue
    is ~600ns/descriptor, so fewer+bigger beats quarter-chunking).

Per-core dataflow (everything "T" = [feature, seq] layout):
  v projection first (own 8-bank psum pool; inputs stream d-ordered in
  64-row chunks split across the sync+scalar DMA issue queues).
  Pipelined head-pair loop (a = 0..7, heads 2a/2a+1 on PE row strips):
    qkproj(a): weight blocks stream just-in-time; q/k tiles rotate (bufs=3)
    scores(a-1): per (s2,c) one [128,1024] fp32 psum tile spanning 2 banks;
      the two heads' K=64 matmuls run concurrently via tile_position row
      strips; ONE fused exp [128,1024] per tile: ScalarE for c=0, custom
      DVE op for c=1 (writes uint16 bits read back as bf16)
    pv(a-2): both heads' PV matmuls run concurrently on column strips
      0-63/64-127 (M=64 each, full 128-row contraction); softmax
      denominators come from four col-tiled ones-matmuls (32-col strips);
      denominators copied to a staging tile (ScalarE), reciprocal on DVE,
      then GpSimd extracts the 4 rows to a bf16 broadcast row
    norm(a-3): GpSimd partition-broadcast + GpSimd multiply (all SBUF)
  fc split ct0-3 (iters 6/7, bf16 partials) / ct4-6 (early drain, in-place
  adds) / ct7 (after the last norm; identity-inject + single eviction).
  Drain norms use PE rank-1 col-tiled broadcasts (ones.T @ recip-row)
  + DVE multiplies (short critical path into the fc7 tail).
  q/k weights arrive host-blocked so each head-pair's column block is one
  contiguous dma (2 issues/iteration instead of 16).
"""

import numpy as np

import concourse.bacc as bacc
import concourse.mybir as mybir
import concourse.tile as tile
from concourse.bass_utils import run_bass_kernel_spmd

S = 1024
D = 1024
H = 16
DK = 64
P = 128
NT = S // P          # 8 seq/feature tiles
NCH = 2              # 512-wide free-dim chunks
CH = S // NCH        # 512
F32 = mybir.dt.float32
BF16 = mybir.dt.bfloat16
U16 = mybir.dt.uint16
EXP = mybir.ActivationFunctionType.Exp

# ---- custom DVE exp constants (see calib: max rel err 0.47%) ----
PRESCALE = 128.0 * 0.125 * np.log2(np.e)   # host multiplies WQ by this
EXP_C1 = 16192.0        # window offset (== 64 mod 128)
EXP_K = 1.5 * 2**30     # magic round-to-128 constant
EXP_CQ = 0.00258        # quadratic mantissa correction
EXP_CC = 54.3           # value offset
EXP_ALPHA = 1.0057634   # resulting common factor; scalar side matches it
SC_SCALE = 0.125 / PRESCALE
SC_BIAS = float(np.log(EXP_ALPHA))

_CACHE = {}


def _register_exp_op():
    import concourse.dve_ops as do
    from concourse.dve_spec import (
        Spec, Src0, C0 as L0, C1 as L1, C2 as L2, C3 as L3, lower,
        _spill_c3_to_src1,
    )
    from concourse.dve_uop import DveOpSpec

    NAME = "EXP2BITS_ANT_X"
    if NAME in do._SUB_OPCODE_FOR_NAME:
        return next(op for op in do.OPS if op.name == NAME)

    Y = Src0 + L0
    T = Y + L1
    R = T - L1
    F = Y - R
    U = F * F
    V = U * L3
    W = Y + V
    body = _spill_c3_to_src1(W + L2)

    def ref(in0, in1, s0, s1, imm2):
        f32 = np.float32
        Pv = in0.astype(f32)
        Yv = (Pv + f32(s0)).astype(f32)
        Tv = (Yv + f32(s1)).astype(f32)
        Rv = (Tv - f32(s1)).astype(f32)
        Fv = (Yv - Rv).astype(f32)
        cq = in1.reshape(in1.shape[0], 1).astype(f32)
        Vv = ((Fv * Fv) * cq).astype(f32)
        return ((Yv + Vv) + f32(imm2)).astype(f32)

    spec = Spec(body=body, reference=ref)
    row = do._CUSTOM_DVE_ROW_BASE + len(do.OPS)
    shas = {}
    for ver in ("v3", "v4"):
        uops = lower(spec, ver=ver)
        shas[ver] = DveOpSpec(name=NAME, opcode=row, uops=uops,
                              rd1_en=True).sha(ver)
    op = do.DveOp(NAME, spec, subdim=False, uops_sha=shas)
    do.OPS.append(op)
    do.CUSTOM_DVE_SPECS[NAME] = spec
    do._SUB_OPCODE_FOR_NAME[NAME] = row
    return op


def _build():
    EXPOP = _register_exp_op()
    nc = bacc.Bacc("TRN2", target_bir_lowering=False, debug=False)
    xqt = nc.dram_tensor("xqt", [D, S], BF16, kind="ExternalInput").ap()
    xkt = nc.dram_tensor("xkt", [D, S], BF16, kind="ExternalInput").ap()
    xvt = nc.dram_tensor("xvt", [D, S], BF16, kind="ExternalInput").ap()
    # q/k weights arrive host-blocked: wqb[a, p, d*128+m] = WQ^T[d*128+p,
    # a*128+m], so each head-pair's column block is one contiguous 2D dma
    wqb = nc.dram_tensor("wqb", [NT, P, D], BF16, kind="ExternalInput").ap()
    wkb = nc.dram_tensor("wkb", [NT, P, D], BF16, kind="ExternalInput").ap()
    wvt = nc.dram_tensor("wvt", [D, D], BF16, kind="ExternalInput").ap()
    wft = nc.dram_tensor("wft", [D, D], BF16, kind="ExternalInput").ap()
    ident = nc.dram_tensor("ident", [P, P], BF16, kind="ExternalInput").ap()
    out = nc.dram_tensor("out", [S, D], BF16, kind="ExternalOutput").ap()

    from contextlib import ExitStack

    with tile.TileContext(nc) as tc:
        with (
            tc.tile_pool(name="persist", bufs=1) as pp,
        ):
            # v natural layout [seq, features]
            vv = [pp.tile([P, D], BF16, tag=f"v{t}", name=f"v{t}")
                  for t in range(NT)]
            ctxT = [pp.tile([P, S], BF16, tag=f"c{t}", name=f"c{t}")
                    for t in range(NT)]
            idt = pp.tile([P, P], BF16, tag="idt", name="idt")
            cqt = pp.tile([P, 1], F32, tag="cqt", name="cqt")
            sbt = pp.tile([P, 1], F32, tag="sbt", name="sbt")

            with ExitStack() as stk:
                ap_ = stk.enter_context(tc.tile_pool(name="attn", bufs=2))
                xtq = [ap_.tile([P, S], BF16, tag="xtq", name="xtq", bufs=8)
                       for _ in range(NT)]
                xtk = [ap_.tile([P, S], BF16, tag="xtk", name="xtk", bufs=8)
                       for _ in range(NT)]
                # fc weights for ct 0-3 get their own slots so the early fc
                # chunks can run while xtq is still live
                wf4 = [ap_.tile([P, S], BF16, tag="wf4", name="wf4", bufs=4)
                       for _ in range(4)]

                # ---- v projection first (attention needs all of v) ----
                with tc.tile_pool(name="vld", bufs=8) as vp, \
                     tc.tile_pool(name="vps", bufs=8, space="PSUM") as vpsp:
                    xts = [vp.tile([P, S], BF16, tag="xt", name="xt")
                           for _ in range(NT)]
                    ws = [vp.tile([P, D], BF16, tag="w", name="w")
                          for _ in range(NT)]
                    # v inputs chunked + d-ordered, split across BOTH
                    # hwdge issue queues (sync + scalar) so they land first;
                    # q/k follow, fc weights last (needed only at iter 6)
                    for t in range(NT):
                        if t < 2:
                            # first tiles: ONE whole-tile descriptor each
                            # (queue issue is ~600ns/descriptor — the
                            # startup is issue-rate bound), spread across
                            # FOUR engine queues so all four transfers are
                            # in flight immediately
                            nc.sync.dma_start(
                                out=xts[t][:],
                                in_=xvt[t * P:(t + 1) * P, :])
                            nc.scalar.dma_start(
                                out=ws[t][:],
                                in_=wvt[t * P:(t + 1) * P, :])
                        else:
                            nc.sync.dma_start(
                                out=xts[t][0:64, :],
                                in_=xvt[t * P:t * P + 64, :])
                            nc.scalar.dma_start(
                                out=xts[t][64:128, :],
                                in_=xvt[t * P + 64:(t + 1) * P, :])
                            nc.sync.dma_start(
                                out=ws[t][0:64, :],
                                in_=wvt[t * P:t * P + 64, :])
                            nc.scalar.dma_start(
                                out=ws[t][64:128, :],
                                in_=wvt[t * P + 64:(t + 1) * P, :])
                    nc.sync.dma_start(out=idt[:], in_=ident)
                    nc.vector.memset(cqt[:], EXP_CQ)
                    nc.vector.memset(sbt[:], SC_BIAS)
                    for t in range(NT):
                        enq = nc.sync if t % 2 == 0 else nc.scalar
                        enk = nc.scalar if t % 2 == 0 else nc.sync
                        enq.dma_start(out=xtq[t][:],
                                      in_=xqt[t * P:(t + 1) * P, :])
                        enk.dma_start(out=xtk[t][:],
                                      in_=xkt[t * P:(t + 1) * P, :])
                    for ct in range(4):
                        nc.scalar.dma_start(out=wf4[ct][:],
                                            in_=wft[ct * P:(ct + 1) * P, :])

                    for s2 in range(NT):
                        pss = [vpsp.tile([P, CH], F32, tag="vp", name="vp")
                               for _ in range(NCH)]
                        for d in range(NT):
                            for c in range(NCH):
                                nc.tensor.matmul(
                                    pss[c][:],
                                    lhsT=xts[d][:, s2 * P:(s2 + 1) * P],
                                    rhs=ws[d][:, c * CH:(c + 1) * CH],
                                    start=(d == 0),
                                    stop=(d == NT - 1),
                                )
                        for c in range(NCH):
                            nc.vector.tensor_copy(
                                vv[s2][:, c * CH:(c + 1) * CH], pss[c][:])

                # second SBUF pool for tags that only exist after the
                # v-load pool is gone (exp tiles, recip rows, fc partials) —
                # keeps the peak footprint under the SBUF limit
                ap2 = stk.enter_context(tc.tile_pool(name="attn2", bufs=2))
                # main psum pool for the attention loop: proj 2 + sc 4 + pv 2
                psp = stk.enter_context(
                    tc.tile_pool(name="psum", bufs=2, space="PSUM"))

                def qkproj(a):
                    # q/k head-pair tiles rotate (lifetime: this iteration's
                    # projection + next iteration's scores); the whole
                    # weight column-block is ONE contiguous dma
                    outs = []
                    for xts_, wsrc, tg in ((xtq, wqb, "qTr"), (xtk, wkb, "kTr")):
                        dst = pp.tile([P, S], BF16, tag=tg, name=tg, bufs=3)
                        wt8 = ap_.tile([P, D], BF16, tag="wqk", name="wqk",
                                       bufs=3)
                        nc.sync.dma_start(out=wt8[:], in_=wsrc[a])
                        pss = [psp.tile([P, CH], F32, tag="ps", name="proj",
                                        bufs=4)
                               for _ in range(NCH)]
                        for d in range(NT):
                            for c in range(NCH):
                                nc.tensor.matmul(
                                    pss[c][:],
                                    lhsT=wt8[:, d * P:(d + 1) * P],
                                    rhs=xts_[d][:, c * CH:(c + 1) * CH],
                                    start=(d == 0),
                                    stop=(d == NT - 1),
                                )
                        for c in range(NCH):
                            # split evictions across DVE / ScalarE
                            if c == 0:
                                nc.vector.tensor_copy(
                                    dst[:, c * CH:(c + 1) * CH], pss[c][:])
                            else:
                                nc.scalar.copy(
                                    dst[:, c * CH:(c + 1) * CH], pss[c][:])
                        outs.append(dst)
                    return outs

                def scores(qk):
                    qTa, kTa = qk
                    # per (s2, c): one fp32 psum tile [128, 1024] spanning 2
                    # banks; the two heads' K=64 matmuls (N=512 each, row
                    # strips 0-63 / 64-127) run concurrently and each fill
                    # one bank; one fused exp [128, 1024] reads both:
                    # ScalarE (c=0) or custom DVE exp2bits (c=1).
                    exps = []
                    for s2 in range(NT):
                        scs = [psp.tile([P, S], F32, tag="sc", name="sc")
                               for _ in range(NCH)]
                        for c in range(NCH):
                            for g in range(2):
                                nc.tensor.matmul(
                                    scs[c][:, g * CH:(g + 1) * CH],
                                    lhsT=kTa[g * DK:(g + 1) * DK,
                                             s2 * P:(s2 + 1) * P],
                                    rhs=qTa[g * DK:(g + 1) * DK,
                                            c * CH:(c + 1) * CH],
                                    start=True, stop=True,
                                    tile_position=(g * DK, 0),
                                )
                        ecs = []
                        for c in range(NCH):
                            ec = ap2.tile([P, S], BF16, tag=f"e{s2}c{c}",
                                          name=f"e{s2}c{c}")
                            if c == 0 or s2 >= NT - 1:
                                nc.scalar.activation(ec[:], scs[c][:], EXP,
                                                     scale=SC_SCALE,
                                                     bias=sbt[0:P, 0:1])
                            else:
                                nc.vector._custom_dve(
                                    EXPOP, out=ec[:].bitcast(U16),
                                    in0=scs[c][:], in1=cqt[:],
                                    s0=EXP_C1, s1=EXP_K, imm2=EXP_CC)
                            ecs.append(ec)
                        exps.append(ecs)
                    return exps

                def pv_phase(a, exps, tail=False):
                    # PV accumulation; ctx_unnorm copied straight out of
                    # psum; the denominator staging feeds DVE reciprocal.
                    # The recip rows stay in stg (fp32) — norm_phase reads
                    # them directly as fp32 rank-1 matmul operands, so no
                    # extract/broadcast chain exists at all.
                    stg = ap2.tile([97, CH], F32, tag="stg", name="stg",
                                   bufs=3)
                    # both heads' PV matmuls run concurrently on column
                    # strips 0-63 / 64-127 of the array (M=64 each, full
                    # 128-row contraction) -> one psum tile holds both
                    pvs = [psp.tile([P, CH], F32, tag="ps", name="pv",
                                    bufs=4)
                           for _ in range(NCH)]
                    # in the drain, scores' psum slots are idle — park the
                    # long-lived denominator bank there instead of starving
                    # the shared rotation
                    if tail:
                        dn = psp.tile([P, S], F32, tag="sc", name="dn",
                                      bufs=2)
                    else:
                        dn = psp.tile([P, CH], F32, tag="ps", name="dn",
                                      bufs=4)
                    for c in range(NCH):
                        for s2 in range(NT):
                            for g in range(2):
                                nc.tensor.matmul(
                                    pvs[c][g * DK:(g + 1) * DK, :],
                                    lhsT=vv[s2][:, (2 * a + g) * DK:
                                                (2 * a + g + 1) * DK],
                                    rhs=exps[s2][c][:, g * CH:(g + 1) * CH],
                                    start=(s2 == 0),
                                    stop=(s2 == NT - 1),
                                    tile_position=(0, g * DK),
                                )
                    # denominators: colsum of each (g,c) exp block via
                    # ones-matmuls on the four 32-col strips
                    for s2 in range(NT):
                        for i in range(4):
                            g, c = i // 2, i % 2
                            nc.tensor.matmul(
                                dn[32 * i:32 * (i + 1), 0:CH],
                                lhsT=ones32[:],
                                rhs=exps[s2][c][:, g * CH:(g + 1) * CH],
                                start=(s2 == 0),
                                stop=(s2 == NT - 1),
                                tile_position=(0, 32 * i),
                            )
                    for c in range(NCH):
                        cp = nc.vector.tensor_copy if c == 0 else \
                            nc.scalar.copy
                        cp(ctxT[a][:, c * CH:(c + 1) * CH], pvs[c][:])
                    # all four denominator rows (at partitions 0/32/64/96)
                    # evacuate in ONE copy (ScalarE; psum-capable) — then
                    # one short reciprocal in place (DVE); stg rows i =
                    # 2*g+c hold the recip for (g, c)
                    nc.vector.tensor_copy(stg[:], dn[0:97, 0:CH])
                    nc.vector.reciprocal_approx_fast(out=stg[:], in_=stg[:])
                    # bf16 copy for the rank-1 broadcast matmuls (fp32
                    # moving operands stream at 1/4 PE rate — bf16 keeps
                    # the psb matmuls at full speed)
                    stgb = ap2.tile([97, CH], BF16, tag="stgb", name="stgb",
                                    bufs=3)
                    nc.vector.tensor_copy(stgb[:], stg[:])
                    return stgb

                def norm_phase(a, stg):
                    # scale ctxT by the recip rows: PE rank-1 col-tiled
                    # fp32 matmuls broadcast each stg row into a psum tile
                    # (both heads), then one DVE multiply per 512-chunk.
                    # Everything rides the PE/DVE queues — no cross-engine
                    # broadcast chain to convoy behind.
                    for c in range(NCH):
                        psb = psp.tile([P, CH], F32, tag="ps",
                                       name="psb", bufs=4)
                        for gg in range(2):
                            i = 2 * gg + c
                            nc.tensor.matmul(
                                psb[gg * DK:(gg + 1) * DK, :],
                                lhsT=ones1f[32 * i:32 * i + 1, 0:DK],
                                rhs=stg[32 * i:32 * i + 1, 0:CH],
                                start=True, stop=True,
                                tile_position=(32 * i, gg * DK),
                            )
                        sl = ctxT[a][:, c * CH:(c + 1) * CH]
                        nc.vector.tensor_mul(sl, sl, psb[:])

                # fc partial tiles (bf16) hold ct0-3, later += ct4-6
                fcp = [ap2.tile([P, CH], BF16, tag=f"fp{i}", name=f"fp{i}",
                                bufs=1)
                       for i in range(2 * NT)]
                ones1f = ap2.tile([97, P], BF16, tag="one1", name="ones1",
                                  bufs=1)
                nc.vector.memset(ones1f[:], 1.0)
                ones32 = ap2.tile([P, 32], BF16, tag="one32", name="ones32",
                                  bufs=1)
                nc.vector.memset(ones32[:], 1.0)

                def fc_chunk(s1, c, cts, wfs, first):
                    pss = psp.tile([P, CH], F32, tag="ps", name="fcc",
                                   bufs=4)
                    for i, ct in enumerate(cts):
                        nc.tensor.matmul(
                            pss[:],
                            lhsT=ctxT[ct][:, s1 * P:(s1 + 1) * P],
                            rhs=wfs[ct][:, c * CH:(c + 1) * CH],
                            start=(i == 0),
                            stop=(i == len(cts) - 1) and first,
                        )
                    if first:
                        nc.vector.tensor_copy(fcp[s1 * NCH + c][:], pss[:])
                    else:
                        # accumulate the existing bf16 partial via identity
                        # matmul (PE has slack here) and evict with a plain
                        # copy, alternating engines — the DVE-add chain was
                        # pacing the drain
                        nc.tensor.matmul(
                            pss[:],
                            lhsT=idt[:],
                            rhs=fcp[s1 * NCH + c][:],
                            start=False, stop=True,
                        )
                        cp = nc.vector.tensor_copy if (s1 + c) % 2 == 0 \
                            else nc.scalar.copy
                        cp(fcp[s1 * NCH + c][:], pss[:])

                exps_hist = None
                qk_hist = None
                r0_hist = {}
                wf = {}
                for a in range(NT):
                    qk_new = qkproj(a)
                    if a == NT - 1:
                        # fc weights for ct 4-7 reuse the q-input slots the
                        # final projection just released; chunked so they
                        # land before the early-drain fc chunks need them
                        for ct in range(4, NT):
                            t_ = ap_.tile([P, S], BF16, tag="xtq", name="wf",
                                          bufs=8)
                            for hh in range(2):
                                nc.sync.dma_start(
                                    out=t_[hh * 64:(hh + 1) * 64, :],
                                    in_=wft[ct * P + hh * 64:
                                            ct * P + (hh + 1) * 64, :])
                            wf[ct] = t_
                    if a >= 2:
                        r0_hist[a - 2] = pv_phase(a - 2, exps_hist)
                    if a >= 1:
                        exps_hist = scores(qk_hist)
                    qk_hist = qk_new
                    if a >= 3:
                        norm_phase(a - 3, r0_hist.pop(a - 3))
                    # fc for ct0-3 interleaves with the last two iterations
                    if a == 6:
                        for s1 in range(4):
                            for c in range(NCH):
                                fc_chunk(s1, c, [0, 1, 2, 3], wf4, True)
                # drain: sc(7) first (its matmuls get early PE queue
                # positions; the second half of the ct0-3 fc pass fills the
                # exp waits); norms 5-6; pv(6); fc ct4-6 split around
                # pv(7); then norm(7); fc ct7 (identity-inject) + store
                exps_last = scores(qk_hist)
                for s1 in range(4, NT):
                    for c in range(NCH):
                        fc_chunk(s1, c, [0, 1, 2, 3], wf4, True)
                norm_phase(NT - 3, r0_hist.pop(NT - 3))
                r0 = pv_phase(NT - 2, exps_hist)
                norm_phase(NT - 2, r0)
                for s1 in range(4):
                    for c in range(NCH):
                        fc_chunk(s1, c, [4, 5, 6], wf, False)
                r0 = pv_phase(NT - 1, exps_last, tail=True)
                for s1 in range(4, NT):
                    for c in range(NCH):
                        fc_chunk(s1, c, [4, 5, 6], wf, False)
                norm_phase(NT - 1, r0)

                for s1 in range(NT):
                    for c in range(NCH):
                        i = s1 * NCH + c
                        pss = psp.tile([P, CH], F32, tag="ps", name="fc7",
                                       bufs=4)
                        nc.tensor.matmul(
                            pss[:],
                            lhsT=ctxT[NT - 1][:, s1 * P:(s1 + 1) * P],
                            rhs=wf[NT - 1][:, c * CH:(c + 1) * CH],
                            start=True, stop=False,
                        )
                        # inject the bf16 partial (ct0-6) via identity
                        # matmul — the PE is idle here, the DVE is not
                        nc.tensor.matmul(
                            pss[:],
                            lhsT=idt[:],
                            rhs=fcp[i][:],
                            start=False, stop=True,
                        )
                        ob = ap_.tile([P, CH], BF16, tag="xtk", name="ob",
                                      bufs=8)
                        # evict in column halves on BOTH engines so each
                        # chunk completes in ~345ns and its store launches
                        # sooner
                        hw = CH // 2
                        nc.vector.tensor_copy(ob[:, 0:hw], pss[:, 0:hw])
                        nc.scalar.copy(ob[:, hw:CH], pss[:, hw:CH])
                        enq = nc.sync if i % 2 == 0 else nc.scalar
                        enq.dma_start(
                            out=out[s1 * P:(s1 + 1) * P,
                                    c * CH:(c + 1) * CH],
                            in_=ob[:],
                        )

    nc.compile()
    return nc


def run(inputs, trace=False):
    """inputs: dict with Q,K,V [8,1024,1024] and WQ,WK,WV,Wfc [1024,1024].
    Returns (out [8,1024,1024] fp32, exec_time_ns or None)."""
    if "nc" not in _CACHE:
        _CACHE["nc"] = _build()
    nc = _CACHE["nc"]

    import ml_dtypes
    bf16 = ml_dtypes.bfloat16
    f32 = np.float32
    def blockw(w, scale=1.0):
        # wb[a, p, d*128+m] = W^T[d*128+p, a*128+m]
        wt = (np.asarray(w, dtype=f32).T * f32(scale)).astype(bf16)
        return np.ascontiguousarray(
            wt.reshape(8, 128, 8, 128).transpose(2, 1, 0, 3).reshape(
                8, 128, 1024))

    wqb = blockw(inputs["WQ"], PRESCALE)
    wkb = blockw(inputs["WK"])
    wvt = np.ascontiguousarray(np.asarray(inputs["WV"], dtype=f32).T.astype(bf16))
    wft = np.ascontiguousarray(np.asarray(inputs["Wfc"], dtype=f32).T.astype(bf16))
    Q = np.asarray(inputs["Q"], dtype=f32)
    K = np.asarray(inputs["K"], dtype=f32)
    V = np.asarray(inputs["V"], dtype=f32)
    ident = np.eye(P, dtype=bf16)

    in_maps = [
        {
            "xqt": np.ascontiguousarray(Q[b].T.astype(bf16)),
            "xkt": np.ascontiguousarray(K[b].T.astype(bf16)),
            "xvt": np.ascontiguousarray(V[b].T.astype(bf16)),
            "wqb": wqb, "wkb": wkb, "wvt": wvt, "wft": wft, "ident": ident,
        }
        for b in range(8)
    ]
    res = run_bass_kernel_spmd(nc, in_maps, core_ids=list(range(8)), trace=trace)
    out = np.stack([res.results[b]["out"] for b in range(8)], axis=0)
    return out.astype(np.float32), res.exec_time_ns


def kernel(**inputs):
    return run(inputs, trace=False)[0]


# revision 51
# speedup vs baseline: 1.1989x; 1.1989x over previous
"""Multi-head attention (B=8, S=1024, D=1024, H=16, dk=dv=64) on 8 TRN2 cores.

Sharding: data-parallel over batch — core b computes batch element b end to
end; no collectives. Host-side prep transposes activations/weights into the
layouts TensorE needs (contraction dim on partitions); all matmuls run on
device in bf16 (fp32 psum accumulate).

Key additions over the previous revision (266us -> 250us):
  * WQ is pre-scaled on host by 16*log2(e), so the scores psum holds
    P = 128*log2(exp-arg) directly. A custom 8-stage DVE op (EXP2BITS)
    converts P to the bf16 BIT PATTERN of exp(score/8)*alpha via a
    Schraudolph-style magic-add (round-to-128 via +-1.5*2^30) with a
    quadratic mantissa correction (max rel err ~0.47%, bf16-grade; out
    dtype uint16 = bf16 bits, round-to-nearest). 7 of 16 exp tiles per
    iteration run on the DVE, relieving the ScalarE activation
    bottleneck (ScalarE handles the rest with a matching *alpha bias —
    exp(x*SC_SCALE + ln(alpha)) — so the common factor cancels in
    softmax normalization). The custom op reads PSUM directly (only
    arithmetic stages — the raw-bit reciprocal op cannot).
  * softmax normalization: recip rows stay in stg; ONE bf16 staging copy,
    then PE rank-1 col+row-tiled matmuls broadcast them into psum and a
    DVE multiply scales ctxT. No gpsimd broadcast / no cross-engine
    convoy (gpsimd tensor ops are slow and library-thrash; a 4-hop
    scalar->DVE->gpsimd->DVE chain was permanently ~1 iteration behind
    and head-of-line blocked the DVE FIFO).
  * fc tail: bf16 fc partials are accumulated into psum via an identity
    matmul (PE has slack in the drain) instead of DVE adds; evictions
    split/alternate DVE+ScalarE; output is stored bf16 (host casts back
    to fp32) halving the output-DMA drain; stores alternate sync/scalar
    queues; fc ct4-6 emission is split around pv(7) as PE filler.
  * startup: first v-tiles as single whole-tile descriptors (queue issue
    is ~600ns/descriptor, so fewer+bigger beats quarter-chunking).

Per-core dataflow (everything "T" = [feature, seq] layout):
  v projection first (own 8-bank psum pool; inputs stream d-ordered in
  64-row chunks split across the sync+scalar DMA issue queues).
  Pipelined head-pair loop (a = 0..7, heads 2a/2a+1 on PE row strips):
    qkproj(a): weight blocks stream just-in-time; q/k tiles rotate (bufs=3)
    scores(a-1): per (s2,c) one [128,1024] fp32 psum tile spanning 2 banks;
      the two heads' K=64 matmuls run concurrently via tile_position row
      strips; ONE fused exp [128,1024] per tile: ScalarE for c=0, custom
      DVE op for c=1 (writes uint16 bits read back as bf16)
    pv(a-2): both heads' PV matmuls run concurrently on column strips
      0-63/64-127 (M=64 each, full 128-row contraction); softmax
      denominators come from four col-tiled ones-matmuls (32-col strips);
      denominators copied to a staging tile (ScalarE), reciprocal on DVE,
      then GpSimd extracts the 4 rows to a bf16 broadcast row
    norm(a-3): GpSimd partition-broadcast + GpSimd multiply (all SBUF)
  fc split ct0-3 (iters 6/7, bf16 partials) / ct4-6 (early drain, in-place
  adds) / ct7 (after the last norm; identity-inject + single eviction).
  Drain norms use PE rank-1 col-tiled broadcasts (ones.T @ recip-row)
  + DVE multiplies (short critical path into the fc7 tail).
  q/k weights arrive host-blocked so each head-pair's column block is one
  contiguous dma (2 issues/iteration instead of 16).
"""

import numpy as np

import concourse.bacc as bacc
import concourse.mybir as mybir
import concourse.tile as tile
from concourse.bass_utils import run_bass_kernel_spmd

S = 1024
D = 1024
H = 16
DK = 64
P = 128
NT = S // P          # 8 seq/feature tiles
NCH = 2              # 512-wide free-dim chunks
CH = S // NCH        # 512
F32 = mybir.dt.float32
BF16 = mybir.dt.bfloat16
U16 = mybir.dt.uint16
EXP = mybir.ActivationFunctionType.Exp

# ---- custom DVE exp constants (see calib: max rel err 0.47%) ----
PRESCALE = 128.0 * 0.125 * np.log2(np.e)   # host multiplies WQ by this
EXP_C1 = 16192.0        # window offset (== 64 mod 128)
EXP_K = 1.5 * 2**30     # magic round-to-128 constant
EXP_CQ = 0.00258        # quadratic mantissa correction
EXP_CC = 54.3           # value offset
EXP_ALPHA = 1.0057634   # resulting common factor; scalar side matches it
SC_SCALE = 0.125 / PRESCALE
SC_BIAS = float(np.log(EXP_ALPHA))

_CACHE = {}


def _register_exp_op():
    import concourse.dve_ops as do
    from concourse.dve_spec import (
        Spec, Src0, C0 as L0, C1 as L1, C2 as L2, C3 as L3, lower,
        _spill_c3_to_src1,
    )
    from concourse.dve_uop import DveOpSpec

    NAME = "EXP2BITS_ANT_X"
    if NAME in do._SUB_OPCODE_FOR_NAME:
        return next(op for op in do.OPS if op.name == NAME)

    Y = Src0 + L0
    T = Y + L1
    R = T - L1
    F = Y - R
    U = F * F
    V = U * L3
    W = Y + V
    body = _spill_c3_to_src1(W + L2)

    def ref(in0, in1, s0, s1, imm2):
        f32 = np.float32
        Pv = in0.astype(f32)
        Yv = (Pv + f32(s0)).astype(f32)
        Tv = (Yv + f32(s1)).astype(f32)
        Rv = (Tv - f32(s1)).astype(f32)
        Fv = (Yv - Rv).astype(f32)
        cq = in1.reshape(in1.shape[0], 1).astype(f32)
        Vv = ((Fv * Fv) * cq).astype(f32)
        return ((Yv + Vv) + f32(imm2)).astype(f32)

    spec = Spec(body=body, reference=ref)
    row = do._CUSTOM_DVE_ROW_BASE + len(do.OPS)
    shas = {}
    for ver in ("v3", "v4"):
        uops = lower(spec, ver=ver)
        shas[ver] = DveOpSpec(name=NAME, opcode=row, uops=uops,
                              rd1_en=True).sha(ver)
    op = do.DveOp(NAME, spec, subdim=False, uops_sha=shas)
    do.OPS.append(op)
    do.CUSTOM_DVE_SPECS[NAME] = spec
    do._SUB_OPCODE_FOR_NAME[NAME] = row
    return op


def _build():
    EXPOP = _register_exp_op()
    nc = bacc.Bacc("TRN2", target_bir_lowering=False, debug=False)
    xqt = nc.dram_tensor("xqt", [D, S], BF16, kind="ExternalInput").ap()
    xkt = nc.dram_tensor("xkt", [D, S], BF16, kind="ExternalInput").ap()
    xvt = nc.dram_tensor("xvt", [D, S], BF16, kind="ExternalInput").ap()
    # q/k weights arrive host-blocked: wqb[a, p, d*128+m] = WQ^T[d*128+p,
    # a*128+m], so each head-pair's column block is one contiguous 2D dma
    wqb = nc.dram_tensor("wqb", [NT, P, D], BF16, kind="ExternalInput").ap()
    wkb = nc.dram_tensor("wkb", [NT, P, D], BF16, kind="ExternalInput").ap()
    wvt = nc.dram_tensor("wvt", [D, D], BF16, kind="ExternalInput").ap()
    wft = nc.dram_tensor("wft", [D, D], BF16, kind="ExternalInput").ap()
    ident = nc.dram_tensor("ident", [P, P], BF16, kind="ExternalInput").ap()
    out = nc.dram_tensor("out", [S, D], BF16, kind="ExternalOutput").ap()

    from contextlib import ExitStack

    with tile.TileContext(nc) as tc:
        with (
            tc.tile_pool(name="persist", bufs=1) as pp,
        ):
            # v natural layout [seq, features]
            vv = [pp.tile([P, D], BF16, tag=f"v{t}", name=f"v{t}")
                  for t in range(NT)]
            ctxT = [pp.tile([P, S], BF16, tag=f"c{t}", name=f"c{t}")
                    for t in range(NT)]
            idt = pp.tile([P, P], BF16, tag="idt", name="idt")
            cqt = pp.tile([P, 1], F32, tag="cqt", name="cqt")
            sbt = pp.tile([P, 1], F32, tag="sbt", name="sbt")

            with ExitStack() as stk:
                ap_ = stk.enter_context(tc.tile_pool(name="attn", bufs=2))
                xtq = [ap_.tile([P, S], BF16, tag="xtq", name="xtq", bufs=8)
                       for _ in range(NT)]
                xtk = [ap_.tile([P, S], BF16, tag="xtk", name="xtk", bufs=8)
                       for _ in range(NT)]
                # fc weights for ct 0-3 get their own slots so the early fc
                # chunks can run while xtq is still live
                wf4 = [ap_.tile([P, S], BF16, tag="wf4", name="wf4", bufs=4)
                       for _ in range(4)]

                # ---- v projection first (attention needs all of v) ----
                with tc.tile_pool(name="vld", bufs=8) as vp, \
                     tc.tile_pool(name="vps", bufs=8, space="PSUM") as vpsp:
                    xts = [vp.tile([P, S], BF16, tag="xt", name="xt")
                           for _ in range(NT)]
                    ws = [vp.tile([P, D], BF16, tag="w", name="w")
                          for _ in range(NT)]
                    # v inputs chunked + d-ordered, split across BOTH
                    # hwdge issue queues (sync + scalar) so they land first;
                    # q/k follow, fc weights last (needed only at iter 6)
                    for t in range(NT):
                        if t < 2:
                            # first tiles: ONE whole-tile descriptor each
                            # (queue issue is ~600ns/descriptor — the
                            # startup is issue-rate bound), spread across
                            # FOUR engine queues so all four transfers are
                            # in flight immediately
                            nc.sync.dma_start(
                                out=xts[t][:],
                                in_=xvt[t * P:(t + 1) * P, :])
                            nc.scalar.dma_start(
                                out=ws[t][:],
                                in_=wvt[t * P:(t + 1) * P, :])
                        else:
                            nc.sync.dma_start(
                                out=xts[t][0:64, :],
                                in_=xvt[t * P:t * P + 64, :])
                            nc.scalar.dma_start(
                                out=xts[t][64:128, :],
                                in_=xvt[t * P + 64:(t + 1) * P, :])
                            nc.sync.dma_start(
                                out=ws[t][0:64, :],
                                in_=wvt[t * P:t * P + 64, :])
                            nc.scalar.dma_start(
                                out=ws[t][64:128, :],
                                in_=wvt[t * P + 64:(t + 1) * P, :])
                    nc.sync.dma_start(out=idt[:], in_=ident)
                    nc.vector.memset(cqt[:], EXP_CQ)
                    nc.vector.memset(sbt[:], SC_BIAS)
                    for t in range(NT):
                        enq = nc.sync if t % 2 == 0 else nc.scalar
                        enk = nc.scalar if t % 2 == 0 else nc.sync
                        enq.dma_start(out=xtq[t][:],
                                      in_=xqt[t * P:(t + 1) * P, :])
                        enk.dma_start(out=xtk[t][:],
                                      in_=xkt[t * P:(t + 1) * P, :])
                    for ct in range(4):
                        nc.scalar.dma_start(out=wf4[ct][:],
                                            in_=wft[ct * P:(ct + 1) * P, :])

                    for s2 in range(NT):
                        pss = [vpsp.tile([P, CH], F32, tag="vp", name="vp")
                               for _ in range(NCH)]
                        for d in range(NT):
                            for c in range(NCH):
                                nc.tensor.matmul(
                                    pss[c][:],
                                    lhsT=xts[d][:, s2 * P:(s2 + 1) * P],
                                    rhs=ws[d][:, c * CH:(c + 1) * CH],
                                    start=(d == 0),
                                    stop=(d == NT - 1),
                                )
                        for c in range(NCH):
                            nc.vector.tensor_copy(
                                vv[s2][:, c * CH:(c + 1) * CH], pss[c][:])

                # second SBUF pool for tags that only exist after the
                # v-load pool is gone (exp tiles, recip rows, fc partials) —
                # keeps the peak footprint under the SBUF limit
                ap2 = stk.enter_context(tc.tile_pool(name="attn2", bufs=2))
                # main psum pool for the attention loop: proj 2 + sc 4 + pv 2
                psp = stk.enter_context(
                    tc.tile_pool(name="psum", bufs=2, space="PSUM"))

                def qkproj(a):
                    # q/k head-pair tiles rotate (lifetime: this iteration's
                    # projection + next iteration's scores); the whole
                    # weight column-block is ONE contiguous dma
                    outs = []
                    for xts_, wsrc, tg in ((xtq, wqb, "qTr"), (xtk, wkb, "kTr")):
                        dst = pp.tile([P, S], BF16, tag=tg, name=tg, bufs=3)
                        wt8 = ap_.tile([P, D], BF16, tag="wqk", name="wqk",
                                       bufs=3)
                        nc.sync.dma_start(out=wt8[:], in_=wsrc[a])
                        pss = [psp.tile([P, CH], F32, tag="ps", name="proj",
                                        bufs=4)
                               for _ in range(NCH)]
                        for d in range(NT):
                            for c in range(NCH):
                                nc.tensor.matmul(
                                    pss[c][:],
                                    lhsT=wt8[:, d * P:(d + 1) * P],
                                    rhs=xts_[d][:, c * CH:(c + 1) * CH],
                                    start=(d == 0),
                                    stop=(d == NT - 1),
                                )
                        for c in range(NCH):
                            # split evictions across DVE / ScalarE
                            if c == 0:
                                nc.vector.tensor_copy(
                                    dst[:, c * CH:(c + 1) * CH], pss[c][:])
                            else:
                                nc.scalar.copy(
                                    dst[:, c * CH:(c + 1) * CH], pss[c][:])
                        outs.append(dst)
                    return outs

                def scores(qk):
                    qTa, kTa = qk
                    # per (s2, c): one fp32 psum tile [128, 1024] spanning 2
                    # banks; the two heads' K=64 matmuls (N=512 each, row
                    # strips 0-63 / 64-127) run concurrently and each fill
                    # one bank; one fused exp [128, 1024] reads both:
                    # ScalarE (c=0) or custom DVE exp2bits (c=1).
                    exps = []
                    for s2 in range(NT):
                        scs = [psp.tile([P, S], F32, tag="sc", name="sc")
                               for _ in range(NCH)]
                        for c in range(NCH):
                            for g in range(2):
                                nc.tensor.matmul(
                                    scs[c][:, g * CH:(g + 1) * CH],
                                    lhsT=kTa[g * DK:(g + 1) * DK,
                                             s2 * P:(s2 + 1) * P],
                                    rhs=qTa[g * DK:(g + 1) * DK,
                                            c * CH:(c + 1) * CH],
                                    start=True, stop=True,
                                    tile_position=(g * DK, 0),
                                )
                        ecs = []
                        for c in range(NCH):
                            ec = ap2.tile([P, S], BF16, tag=f"e{s2}c{c}",
                                          name=f"e{s2}c{c}")
                            if c == 0 or s2 >= NT - 1:
                                nc.scalar.activation(ec[:], scs[c][:], EXP,
                                                     scale=SC_SCALE,
                                                     bias=sbt[0:P, 0:1])
                            else:
                                nc.vector._custom_dve(
                                    EXPOP, out=ec[:].bitcast(U16),
                                    in0=scs[c][:], in1=cqt[:],
                                    s0=EXP_C1, s1=EXP_K, imm2=EXP_CC)
                            ecs.append(ec)
                        exps.append(ecs)
                    return exps

                def pv_phase(a, exps, tail=False):
                    # PV accumulation; ctx_unnorm copied straight out of
                    # psum; the denominator staging feeds DVE reciprocal.
                    # The recip rows stay in stg (fp32) — norm_phase reads
                    # them directly as fp32 rank-1 matmul operands, so no
                    # extract/broadcast chain exists at all.
                    stg = ap2.tile([97, CH], F32, tag="stg", name="stg",
                                   bufs=3)
                    # both heads' PV matmuls run concurrently on column
                    # strips 0-63 / 64-127 of the array (M=64 each, full
                    # 128-row contraction) -> one psum tile holds both
                    pvs = [psp.tile([P, CH], F32, tag="ps", name="pv",
                                    bufs=4)
                           for _ in range(NCH)]
                    # in the drain, scores' psum slots are idle — park the
                    # long-lived denominator bank there instead of starving
                    # the shared rotation
                    if tail:
                        dn = psp.tile([P, S], F32, tag="sc", name="dn",
                                      bufs=2)
                    else:
                        dn = psp.tile([P, CH], F32, tag="ps", name="dn",
                                      bufs=4)
                    for c in range(NCH):
                        for s2 in range(NT):
                            for g in range(2):
                                nc.tensor.matmul(
                                    pvs[c][g * DK:(g + 1) * DK, :],
                                    lhsT=vv[s2][:, (2 * a + g) * DK:
                                                (2 * a + g + 1) * DK],
                                    rhs=exps[s2][c][:, g * CH:(g + 1) * CH],
                                    start=(s2 == 0),
                                    stop=(s2 == NT - 1),
                                    tile_position=(0, g * DK),
                                )
                    # denominators: colsum of each (g,c) exp block via
                    # ones-matmuls on the four 32-col strips
                    for s2 in range(NT):
                        for i in range(4):
                            g, c = i // 2, i % 2
                            nc.tensor.matmul(
                                dn[32 * i:32 * (i + 1), 0:CH],
                                lhsT=ones32[:],
                                rhs=exps[s2][c][:, g * CH:(g + 1) * CH],
                                start=(s2 == 0),
                                stop=(s2 == NT - 1),
                                tile_position=(0, 32 * i),
                            )
                    for c in range(NCH):
                        cp = nc.vector.tensor_copy if c == 0 else \
                            nc.scalar.copy
                        cp(ctxT[a][:, c * CH:(c + 1) * CH], pvs[c][:])
                    # all four denominator rows (at partitions 0/32/64/96)
                    # evacuate in ONE copy (ScalarE; psum-capable) — then
                    # one short reciprocal in place (DVE); stg rows i =
                    # 2*g+c hold the recip for (g, c)
                    nc.scalar.copy(stg[:], dn[0:97, 0:CH])
                    nc.vector.reciprocal_approx_fast(out=stg[:], in_=stg[:])
                    # bf16 copy for the rank-1 broadcast matmuls (fp32
                    # moving operands stream at 1/4 PE rate — bf16 keeps
                    # the psb matmuls at full speed)
                    stgb = ap2.tile([97, CH], BF16, tag="stgb", name="stgb",
                                    bufs=3)
                    nc.vector.tensor_copy(stgb[:], stg[:])
                    return stgb

                def norm_phase(a, stg):
                    # scale ctxT by the recip rows: PE rank-1 col-tiled
                    # fp32 matmuls broadcast each stg row into a psum tile
                    # (both heads), then one DVE multiply per 512-chunk.
                    # Everything rides the PE/DVE queues — no cross-engine
                    # broadcast chain to convoy behind.
                    for c in range(NCH):
                        psb = psp.tile([P, CH], F32, tag="ps",
                                       name="psb", bufs=4)
                        for gg in range(2):
                            i = 2 * gg + c
                            nc.tensor.matmul(
                                psb[gg * DK:(gg + 1) * DK, :],
                                lhsT=ones1f[32 * i:32 * i + 1, 0:DK],
                                rhs=stg[32 * i:32 * i + 1, 0:CH],
                                start=True, stop=True,
                                tile_position=(32 * i, gg * DK),
                            )
                        sl = ctxT[a][:, c * CH:(c + 1) * CH]
                        nc.vector.tensor_mul(sl, sl, psb[:])

                # fc partial tiles (bf16) hold ct0-3, later += ct4-6
                fcp = [ap2.tile([P, CH], BF16, tag=f"fp{i}", name=f"fp{i}",
                                bufs=1)
                       for i in range(2 * NT)]
                ones1f = ap2.tile([97, P], BF16, tag="one1", name="ones1",
                                  bufs=1)
                nc.vector.memset(ones1f[:], 1.0)
                ones32 = ap2.tile([P, 32], BF16, tag="one32", name="ones32",
                                  bufs=1)
                nc.vector.memset(ones32[:], 1.0)

                def fc_chunk(s1, c, cts, wfs, first):
                    pss = psp.tile([P, CH], F32, tag="ps", name="fcc",
                                   bufs=4)
                    for i, ct in enumerate(cts):
                        nc.tensor.matmul(
                            pss[:],
                            lhsT=ctxT[ct][:, s1 * P:(s1 + 1) * P],
                            rhs=wfs[ct][:, c * CH:(c + 1) * CH],
                            start=(i == 0),
                            stop=(i == len(cts) - 1) and first,
                        )
                    if first:
                        nc.vector.tensor_copy(fcp[s1 * NCH + c][:], pss[:])
                    else:
                        # accumulate the existing bf16 partial via identity
                        # matmul (PE has slack here) and evict with a plain
                        # copy, alternating engines — the DVE-add chain was
                        # pacing the drain
                        nc.tensor.matmul(
                            pss[:],
                            lhsT=idt[:],
                            rhs=fcp[s1 * NCH + c][:],
                            start=False, stop=True,
                        )
                        cp = nc.vector.tensor_copy if (s1 + c) % 2 == 0 \
                            else nc.scalar.copy
                        cp(fcp[s1 * NCH + c][:], pss[:])

                exps_hist = None
                qk_hist = None
                r0_hist = {}
                wf = {}
                for a in range(NT):
                    qk_new = qkproj(a)
                    if a == NT - 1:
                        # fc weights for ct 4-7 reuse the q-input slots the
                        # final projection just released; chunked so they
                        # land before the early-drain fc chunks need them
                        for ct in range(4, NT):
                            t_ = ap_.tile([P, S], BF16, tag="xtq", name="wf",
                                          bufs=8)
                            for hh in range(2):
                                nc.sync.dma_start(
                                    out=t_[hh * 64:(hh + 1) * 64, :],
                                    in_=wft[ct * P + hh * 64:
                                            ct * P + (hh + 1) * 64, :])
                            wf[ct] = t_
                    if a >= 2:
                        r0_hist[a - 2] = pv_phase(a - 2, exps_hist)
                    if a >= 1:
                        exps_hist = scores(qk_hist)
                    qk_hist = qk_new
                    if a >= 3:
                        norm_phase(a - 3, r0_hist.pop(a - 3))
                    # fc for ct0-3 interleaves with the last two iterations
                    if a == 6:
                        for s1 in range(4):
                            for c in range(NCH):
                                fc_chunk(s1, c, [0, 1, 2, 3], wf4, True)
                # drain: sc(7) first (its matmuls get early PE queue
                # positions; the second half of the ct0-3 fc pass fills the
                # exp waits); norms 5-6; pv(6); fc ct4-6 split around
                # pv(7); then norm(7); fc ct7 (identity-inject) + store
                exps_last = scores(qk_hist)
                for s1 in range(4, NT):
                    for c in range(NCH):
                        fc_chunk(s1, c, [0, 1, 2, 3], wf4, True)
                norm_phase(NT - 3, r0_hist.pop(NT - 3))
                r0 = pv_phase(NT - 2, exps_hist)
                norm_phase(NT - 2, r0)
                for s1 in range(4):
                    for c in range(NCH):
                        fc_chunk(s1, c, [4, 5, 6], wf, False)
                r0 = pv_phase(NT - 1, exps_last, tail=True)
                for s1 in range(4, NT):
                    for c in range(NCH):
                        fc_chunk(s1, c, [4, 5, 6], wf, False)
                norm_phase(NT - 1, r0)

                for s1 in range(NT):
                    for c in range(NCH):
                        i = s1 * NCH + c
                        pss = psp.tile([P, CH], F32, tag="ps", name="fc7",
                                       bufs=4)
                        nc.tensor.matmul(
                            pss[:],
                            lhsT=ctxT[NT - 1][:, s1 * P:(s1 + 1) * P],
                            rhs=wf[NT - 1][:, c * CH:(c + 1) * CH],
                            start=True, stop=False,
                        )
                        # inject the bf16 partial (ct0-6) via identity
                        # matmul — the PE is idle here, the DVE is not
                        nc.tensor.matmul(
                            pss[:],
                            lhsT=idt[:],
                            rhs=fcp[i][:],
                            start=False, stop=True,
                        )
                        ob = ap_.tile([P, CH], BF16, tag="xtk", name="ob",
                                      bufs=8)
                        # evict in column halves on BOTH engines so each
                        # chunk completes in ~345ns and its store launches
                        # sooner
                        hw = CH // 2
                        nc.vector.tensor_copy(ob[:, 0:hw], pss[:, 0:hw])
                        nc.scalar.copy(ob[:, hw:CH], pss[:, hw:CH])
                        enq = nc.sync if i % 2 == 0 else nc.scalar
                        enq.dma_start(
                            out=out[s1 * P:(s1 + 1) * P,
                                    c * CH:(c + 1) * CH],
                            in_=ob[:],
                        )

    nc.compile()
    return nc


def run(inputs, trace=False):
    """inputs: dict with Q,K,V [8,1024,1024] and WQ,WK,WV,Wfc [1024,1024].
    Returns (out [8,1024,1024] fp32, exec_time_ns or None)."""
    if "nc" not in _CACHE:
        _CACHE["nc"] = _build()
    nc = _CACHE["nc"]

    import ml_dtypes
    bf16 = ml_dtypes.bfloat16
    f32 = np.float32
    def blockw(w, scale=1.0):
        # wb[a, p, d*128+m] = W^T[d*128+p, a*128+m]
        wt = (np.asarray(w, dtype=f32).T * f32(scale)).astype(bf16)
        return np.ascontiguousarray(
            wt.reshape(8, 128, 8, 128).transpose(2, 1, 0, 3).reshape(
                8, 128, 1024))

    wqb = blockw(inputs["WQ"], PRESCALE)
    wkb = blockw(inputs["WK"])
    wvt = np.ascontiguousarray(np.asarray(inputs["WV"], dtype=f32).T.astype(bf16))
    wft = np.ascontiguousarray(np.asarray(inputs["Wfc"], dtype=f32).T.astype(bf16))
    Q = np.asarray(inputs["Q"], dtype=f32)
    K = np.asarray(inputs["K"], dtype=f32)
    V = np.asarray(inputs["V"], dtype=f32)
    ident = np.eye(P, dtype=bf16)

    in_maps = [
        {
            "xqt": np.ascontiguousarray(Q[b].T.astype(bf16)),
            "xkt": np.ascontiguousarray(K[b].T.astype(bf16)),
            "xvt": np.ascontiguousarray(V[b].T.astype(bf16)),
            "wqb": wqb, "wkb": wkb, "wvt": wvt, "wft": wft, "ident": ident,
        }
        for b in range(8)
    ]
    res = run_bass_kernel_spmd(nc, in_maps, core_ids=list(range(8)), trace=trace)
    out = np.stack([res.results[b]["out"] for b in range(8)], axis=0)
    return out.astype(np.float32), res.exec_time_ns


def kernel(**inputs):
    return run(inputs, trace=False)[0]
